# revision 10
# baseline (speedup 1.0000x reference)
"""Trainium2 kernel for nn_Attend_13537736916998 (sparse_attention).

Mathematical reduction of the reference:
  - sim <= 0 everywhere, so the selective-attention gate relu(sim[:, 0]) is
    identically zero -> the gate/cumsum branch is a numerical no-op.
  - attn = hard + soft - stop_gradient(soft) evaluates elementwise to the
    one-hot `hard` (+ O(2^-24)).  Hence
    out[b,h,i,:] = v[b,h, argmax_{j<=i} (q_i.k_j - 0.5||k_j||^2), :].

Score matmul: exact-enough 2-pass fp16 limb decomposition (1 cyc/row/pass):
  pass1: [qhi; qlo]^T @ [khi; khi]   = (qhi+qlo).khi
  pass2: [qhi; 1; 1]^T @ [klo; b1; b2] = qhi.klo + b      (b = -0.5||k||^2)
plus a third tiny matmul on the diagonal 128-block adding -60000*[j > i]
(tri^T @ -60000*I) which implements the causal mask inside PSUM.
Verified: 0/32768 rows differ from the fp32 reference argmax.

Argmax pipeline per 128-row tile (scores stay in PSUM, never copied):
  - max pass: vector max8 or gpsimd reduce_max per <=1024-col segment,
    tiny combine -> gmax[P,1]
  - scalar engine: onehot16 = exp(2^100*S - 2^100*gmax)  -- exactly 1.0 at
    the argmax (2^100 scaling is exact: power-of-two), 0 elsewhere
    (any gap >= 1 ulp underflows exp to 0)
  - vector STT at fp16 2x: idx = sum(onehot16 * iota16)  (indices < 2048 are
    exact in fp16)
  - gpsimd indirect DMA gathers the winning v rows from HBM.

Output is emitted in gather layout [2, 128, 16, 64] (partition-major) and
re-ordered on the host during unsharding.
"""

import numpy as np
from contextlib import ExitStack

import concourse.bass as bass
import concourse.bacc as bacc
import concourse.tile as tile
from concourse import mybir
import concourse.bass_utils as bass_utils

B, H, N, D = 2, 8, 2048, 64
P = 128
NT = N // P            # 16 row tiles per (b,h) pair
T = 2                  # (b,h) pairs per core
NCORES = 8
F32 = mybir.dt.float32
F16 = mybir.dt.float16
U32 = mybir.dt.uint32
C100 = float(2.0 ** 100)   # exact power-of-two softargmax scale
MASKVAL = -60000.0         # fp16-representable; dwarfs any valid score

# gpsimd.tensor_reduce only reduces along the partition axis, so the max
# pass cannot be offloaded there; all row maxes run on the vector engine.
GPSIMD_MAX_TILES = frozenset()


def kernel_body(tc, qa, kt, v, out):
    nc = tc.nc
    with ExitStack() as ctx:
        consts = ctx.enter_context(tc.tile_pool(name="consts", bufs=1))
        io = ctx.enter_context(tc.tile_pool(name="io", bufs=2))
        work = ctx.enter_context(tc.tile_pool(name="work", bufs=4))
        outp = ctx.enter_context(tc.tile_pool(name="outp", bufs=2))
        small = ctx.enter_context(tc.tile_pool(name="small", bufs=8))
        ps_pool = ctx.enter_context(tc.tile_pool(name="ps", bufs=3, space="PSUM"))
        psk_pool = ctx.enter_context(tc.tile_pool(name="psk", bufs=1, space="PSUM"))

        ones_col = consts.tile([D, 1], F32)
        nc.vector.memset(ones_col, 1.0)
        # iota row replicated on every partition, fp16 (0..2047 exact)
        iota16 = consts.tile([P, N], F16)
        nc.gpsimd.iota(iota16, pattern=[[1, N]], base=0, channel_multiplier=0,
                       allow_small_or_imprecise_dtypes=True)
        # causal-mask matmul constants: tri[d,i] = 1[d > i]; negI = MASKVAL*I
        triA = consts.tile([P, P], F16)
        nc.vector.memset(triA, 1.0)
        nc.gpsimd.affine_select(out=triA, in_=triA, pattern=[[-1, P]], base=-1,
                                channel_multiplier=1,
                                compare_op=mybir.AluOpType.is_ge, fill=0.0)
        negI = consts.tile([P, P], F16)
        nc.vector.memset(negI, MASKVAL)
        nc.gpsimd.affine_select(out=negI, in_=negI, pattern=[[-1, P]], base=0,
                                channel_multiplier=1,
                                compare_op=mybir.AluOpType.is_equal, fill=0.0)

        for t in range(T):
            # ---- q-side prep: fp32 load + fp16 limb split, high cols first
            qa_t = io.tile([D + 1, N], F32, tag="qa")
            qhl = io.tile([2 * D, N], F16, tag="qhl")      # [qhi; qlo]
            qho = io.tile([D + 2, N], F16, tag="qho")      # [qhi; 1; 1]
            nc.vector.memset(qho[D:D + 2, :], 1.0)
            for c in reversed(range(N // 512)):
                cs = slice(c * 512, (c + 1) * 512)
                nc.sync.dma_start(out=qa_t[:, cs], in_=qa[t][:, cs])
                nc.scalar.copy(qhl[0:D, cs], qa_t[0:D, cs])            # qhi
                nc.gpsimd.tensor_sub(qhl[D:2 * D, cs], qa_t[0:D, cs],
                                     qhl[0:D, cs])                     # qlo
                nc.scalar.copy(qho[0:D, cs], qa_t[0:D, cs])            # qhi dup

            # ---- k-side prep: fp16 limbs + fp32 ksq bias (split to fp16)
            kt_t = io.tile([D, N], F32, tag="kt")
            sq = io.tile([D, N], F32, tag="sq")
            khh = io.tile([2 * D, N], F16, tag="khh")      # [khi; khi]
            klb = io.tile([D + 2, N], F16, tag="klb")      # [klo; b1; b2]
            b32 = io.tile([1, N], F32, tag="b32")
            bb = io.tile([1, 2, N], F16, tag="bb")         # staged bias limbs
            for c in range(N // 512):
                cs = slice(c * 512, (c + 1) * 512)
                nc.sync.dma_start(out=kt_t[:, cs], in_=kt[t][:, cs])
                nc.scalar.copy(khh[0:D, cs], kt_t[:, cs])              # khi
                nc.gpsimd.tensor_sub(klb[0:D, cs], kt_t[:, cs],
                                     khh[0:D, cs])                     # klo
                nc.scalar.copy(khh[D:2 * D, cs], kt_t[:, cs])          # khi dup
                nc.scalar.square(sq[:, cs], kt_t[:, cs])
                pk = psk_pool.tile([1, 512], F32, tag="pk")
                nc.tensor.matmul(pk, lhsT=ones_col, rhs=sq[:, cs],
                                 start=True, stop=True)
                nc.scalar.mul(b32[:, cs], pk, -0.5)
                nc.scalar.copy(bb[:, 0, cs], b32[:, cs])               # b1
                nc.vector.tensor_sub(bb[:, 1, cs], b32[:, cs],
                                     bb[:, 0, cs])                     # b2
                nc.sync.dma_start(out=klb[D:D + 2, cs], in_=bb[:, :, cs])

            idxs = outp.tile([P, NT, 8], U32, tag="idxs")
            vout = outp.tile([P, NT, D], F32, tag="vout")
            # big/small interleave keeps the PE fed while scans drain.
            order = []
            lo_m, hi_m = 0, NT - 1
            while hi_m >= lo_m:
                order.append(hi_m); hi_m -= 1
                if hi_m >= lo_m:
                    order.append(lo_m); lo_m += 1
            for m in order:
                W = (m + 1) * P
                ms = slice(m * P, (m + 1) * P)
                on_gp = m in GPSIMD_MAX_TILES
                nseg = (W + 1023) // 1024
                oh16 = work.tile([P, N], F16, tag="oh")    # exp one-hot
                jk16 = work.tile([P, N], F16, tag="jk")    # STT scratch out
                smax = small.tile([P, 2], F32, tag="smax")
                m8 = small.tile([P, 2, 8], F32, tag="m8")
                acc = small.tile([P, 2], F32, tag="acc")
                gm = small.tile([P, 1], F32, tag="gm")
                gmc = small.tile([P, 1], F32, tag="gmc")
                segs = []
                for s in range(nseg):
                    slo = s * 1024
                    shi = min(W, slo + 1024)
                    sw = shi - slo
                    ps = ps_pool.tile([P, 1024], F32, tag="ps")
                    segs.append((ps, slo, sw))
                    for lo in range(slo, shi, 512):
                        hi = min(shi, lo + 512)
                        pslo = lo - slo
                        pshi = hi - slo
                        nc.tensor.matmul(ps[:, pslo:pshi], lhsT=qhl[:, ms],
                                         rhs=khh[:, lo:hi],
                                         start=True, stop=False)
                        if hi == W:
                            # causal mask on the diagonal 128 cols, via PE:
                            # adds MASKVAL*[j > i] mid-accumulation-group.
                            dlo = (W - P) - slo
                            nc.tensor.matmul(ps[:, dlo:dlo + P], lhsT=triA,
                                             rhs=negI, start=False, stop=False)
                        nc.tensor.matmul(ps[:, pslo:pshi], lhsT=qho[:, ms],
                                         rhs=klb[:, lo:hi],
                                         start=False, stop=True)
                    # max pass for this segment
                    if on_gp:
                        nc.gpsimd.tensor_reduce(smax[:, s:s + 1],
                                                ps[:, 0:sw],
                                                axis=mybir.AxisListType.X,
                                                op=mybir.AluOpType.max)
                    else:
                        nc.vector.max(m8[:, s, :], ps[:, 0:sw])
                # combine segment maxes -> gmc = -2^100 * gmax
                if on_gp:
                    if nseg == 2:
                        nc.gpsimd.tensor_max(gm, smax[:, 0:1], smax[:, 1:2])
                        gsrc = gm
                    else:
                        gsrc = smax[:, 0:1]
                else:
                    if nseg == 2:
                        nc.vector.tensor_max(gm, m8[:, 0, 0:1], m8[:, 1, 0:1])
                        gsrc = gm
                    else:
                        gsrc = m8[:, 0, 0:1]
                nc.scalar.mul(gmc, gsrc, -C100)
                # one-hot + iota-sum index extraction per segment
                for s, (ps, slo, sw) in enumerate(segs):
                    nc.scalar.activation(oh16[:, slo:slo + sw], ps[:, 0:sw],
                                         mybir.ActivationFunctionType.Exp,
                                         bias=gmc, scale=C100)
                    nc.vector.scalar_tensor_tensor(
                        out=jk16[:, slo:slo + sw],
                        in0=oh16[:, slo:slo + sw],
                        scalar=1.0,
                        in1=iota16[:, slo:slo + sw],
                        op0=mybir.AluOpType.bypass,
                        op1=mybir.AluOpType.mult,
                        accum_out=acc[:, s:s + 1],
                    )
                if nseg == 2:
                    nc.vector.tensor_add(acc[:, 0:1], acc[:, 0:1], acc[:, 1:2])
                nc.vector.tensor_copy(idxs[:, m, 0:1], acc[:, 0:1])  # fp32->u32
                # gather the 128 winning v rows for this row tile.
                # NB: one offset column per indirect DMA — multi-column offset
                # tables mis-generate descriptors on HW.
                nc.gpsimd.indirect_dma_start(
                    out=vout[:, m, :],
                    out_offset=None,
                    in_=v,
                    in_offset=bass.IndirectOffsetOnAxis(ap=idxs[:, m, 0:1], axis=1),
                    element_offset=t * N * D,
                )

            nc.sync.dma_start(out=out[t][:, 8:NT, :], in_=vout[:, 8:NT, :])
            nc.sync.dma_start(out=out[t][:, 0:8, :], in_=vout[:, 0:8, :])


_NC_CACHE = None


def build_nc():
    global _NC_CACHE
    if _NC_CACHE is not None:
        return _NC_CACHE
    nc = bacc.Bacc(
        "TRN2",
        target_bir_lowering=False,
        debug=False,
        enable_asserts=False,
        num_devices=NCORES,
    )
    qa = nc.dram_tensor("qa", [T, D + 1, N], F32, kind="ExternalInput").ap()
    kt = nc.dram_tensor("kt", [T, D, N], F32, kind="ExternalInput").ap()
    v = nc.dram_tensor("v", [T, N, D], F32, kind="ExternalInput").ap()
    out = nc.dram_tensor("out", [T, P, NT, D], F32, kind="ExternalOutput").ap()
    with tile.TileContext(nc) as tc:
        kernel_body(tc, qa, kt, v, out)
    nc.compile()
    _NC_CACHE = nc
    return nc


def make_in_maps(q, k, v):
    q = np.asarray(q, dtype=np.float32)
    k = np.asarray(k, dtype=np.float32)
    v = np.asarray(v, dtype=np.float32)
    assert q.shape == (B, H, N, D), q.shape
    in_maps = []
    for c in range(NCORES):
        qa_c = np.empty((T, D + 1, N), np.float32)
        kt_c = np.empty((T, D, N), np.float32)
        v_c = np.empty((T, N, D), np.float32)
        for t in range(T):
            gp = T * c + t
            b, h = divmod(gp, H)
            qa_c[t, :D] = q[b, h].T
            qa_c[t, D] = 1.0
            kt_c[t] = k[b, h].T
            v_c[t] = v[b, h]
        in_maps.append({"qa": qa_c, "kt": kt_c, "v": v_c})
    return in_maps


def unmarshal(results):
    out = np.empty((B, H, N, D), np.float32)
    for c in range(NCORES):
        o = np.asarray(results[c]["out"])  # [T, P, NT, D]
        for t in range(T):
            gp = T * c + t
            b, h = divmod(gp, H)
            out[b, h] = o[t].transpose(1, 0, 2).reshape(N, D)
    return out


def kernel(q, k, v):
    nc = build_nc()
    in_maps = make_in_maps(q, k, v)
    res = bass_utils.run_bass_kernel_spmd(nc, in_maps, core_ids=list(range(NCORES)))
    return unmarshal(res.results)


# revision 11
# speedup vs baseline: 1.2208x; 1.2208x over previous
"""Trainium2 kernel for nn_Attend_13537736916998 (sparse_attention).

Mathematical reduction of the reference:
  - sim <= 0 everywhere, so the selective-attention gate relu(sim[:, 0]) is
    identically zero -> the gate/cumsum branch is a numerical no-op.
  - attn = hard + soft - stop_gradient(soft) evaluates elementwise to the
    one-hot `hard` (+ O(2^-24)).  Hence
    out[b,h,i,:] = v[b,h, argmax_{j<=i} (q_i.k_j - 0.5||k_j||^2), :].

Score matmul: exact-enough 2-pass fp16 limb decomposition (1 cyc/row/pass
on the PE instead of fp32's 4):
  pass1: [qhi; qlo]^T @ [khi; khi]   = (qhi+qlo).khi
  pass2: [qhi; 1; 1]^T @ [klo; b1; b2] = qhi.klo + b      (b = -0.5||k||^2)
plus a third tiny matmul on the diagonal 128-block adding -60000*[j > i]
(tri^T @ (-60000*I)), which implements the causal mask inside PSUM and keeps
the mask off the vector/gpsimd critical path.
Verified: 0/32768 output rows differ from the fp32 reference argmax.

Per 128-row tile: scalar engine copies PSUM chunks to SBUF, vector max8 +
find_index8 give the causal argmax, gpsimd indirect DMA gathers the winning
v rows from HBM.  fp16 limb subtractions run on gpsimd; dtype-dup rows are
materialized by SBUF-to-SBUF DMA to keep the vector engine on scans only.

Output is emitted in gather layout [2, 128, 16, 64] (partition-major) and
re-ordered on the host during unsharding.
"""

import numpy as np
from contextlib import ExitStack

import concourse.bass as bass
import concourse.bacc as bacc
import concourse.tile as tile
from concourse import mybir
import concourse.bass_utils as bass_utils

B, H, N, D = 2, 8, 2048, 64
P = 128
NT = N // P            # 16 row tiles per (b,h) pair
T = 2                  # (b,h) pairs per core
NCORES = 8
F32 = mybir.dt.float32
F16 = mybir.dt.float16
U32 = mybir.dt.uint32
MASKVAL = -60000.0     # fp16-representable; dwarfs any valid score


def kernel_body(tc, qa, kt, v, out):
    nc = tc.nc
    with ExitStack() as ctx:
        consts = ctx.enter_context(tc.tile_pool(name="consts", bufs=1))
        io = ctx.enter_context(tc.tile_pool(name="io", bufs=2))
        work = ctx.enter_context(tc.tile_pool(name="work", bufs=4))
        outp = ctx.enter_context(tc.tile_pool(name="outp", bufs=2))
        small = ctx.enter_context(tc.tile_pool(name="small", bufs=6))
        ps_pool = ctx.enter_context(tc.tile_pool(name="ps", bufs=7, space="PSUM"))
        psk_pool = ctx.enter_context(tc.tile_pool(name="psk", bufs=1, space="PSUM"))

        ones_col = consts.tile([D, 1], F32)
        nc.vector.memset(ones_col, 1.0)
        ones2 = consts.tile([1, 2, N], F16)     # staged [1;1] rows for qho
        nc.vector.memset(ones2, 1.0)
        # causal-mask matmul constants: tri[d,i] = 1[d > i]; negI = MASKVAL*I
        triA = consts.tile([P, P], F16)
        nc.vector.memset(triA, 1.0)
        nc.gpsimd.affine_select(out=triA, in_=triA, pattern=[[-1, P]], base=-1,
                                channel_multiplier=1,
                                compare_op=mybir.AluOpType.is_ge, fill=0.0)
        negI = consts.tile([P, P], F16)
        nc.vector.memset(negI, MASKVAL)
        nc.gpsimd.affine_select(out=negI, in_=negI, pattern=[[-1, P]], base=0,
                                channel_multiplier=1,
                                compare_op=mybir.AluOpType.is_equal, fill=0.0)

        for t in range(T):
            # ---- q-side prep: fp32 load + fp16 limb split, high cols first
            # (row tiles run largest-first, so high lhsT slices are needed
            # first).
            qa_t = io.tile([D + 1, N], F32, tag="qa")
            qhl = io.tile([2 * D, N], F16, tag="qhl")      # [qhi; qlo]
            qho = io.tile([D + 2, N], F16, tag="qho")      # [qhi; 1; 1]
            nc.sync.dma_start(out=qho[D:D + 2, :], in_=ones2[:, :, :])
            for c in reversed(range(N // 512)):
                cs = slice(c * 512, (c + 1) * 512)
                nc.sync.dma_start(out=qa_t[:, cs], in_=qa[t][:, cs])
                nc.scalar.copy(qhl[0:D, cs], qa_t[0:D, cs])            # qhi
                nc.gpsimd.tensor_sub(qhl[D:2 * D, cs], qa_t[0:D, cs],
                                     qhl[0:D, cs])                     # qlo
                nc.sync.dma_start(out=qho[0:D, cs], in_=qhl[0:D, cs])  # qhi dup

            # ---- k-side prep: fp16 limbs + fp32 ksq bias (split to fp16)
            kt_t = io.tile([D, N], F32, tag="kt")
            sq = io.tile([D, N], F32, tag="sq")
            khh = io.tile([2 * D, N], F16, tag="khh")      # [khi; khi]
            klb = io.tile([D + 2, N], F16, tag="klb")      # [klo; b1; b2]
            b32 = io.tile([1, N], F32, tag="b32")
            # bias limbs staged on partition 0 (engine writes must start at a
            # multiple-of-32 partition), then DMA'd into klb partitions 64:66
            bb = io.tile([1, 2, N], F16, tag="bb")
            for c in range(N // 512):
                cs = slice(c * 512, (c + 1) * 512)
                nc.sync.dma_start(out=kt_t[:, cs], in_=kt[t][:, cs])
                nc.scalar.copy(khh[0:D, cs], kt_t[:, cs])              # khi
                nc.gpsimd.tensor_sub(klb[0:D, cs], kt_t[:, cs],
                                     khh[0:D, cs])                     # klo
                nc.sync.dma_start(out=khh[D:2 * D, cs], in_=khh[0:D, cs])
                nc.scalar.square(sq[:, cs], kt_t[:, cs])
                pk = psk_pool.tile([1, 512], F32, tag="pk")
                nc.tensor.matmul(pk, lhsT=ones_col, rhs=sq[:, cs],
                                 start=True, stop=True)
                nc.scalar.mul(b32[:, cs], pk, -0.5)
                nc.scalar.copy(bb[:, 0, cs], b32[:, cs])               # b1
                nc.vector.tensor_sub(bb[:, 1, cs], b32[:, cs],
                                     bb[:, 0, cs])                     # b2
                nc.sync.dma_start(out=klb[D:D + 2, cs], in_=bb[:, :, cs])

            idxs = outp.tile([P, NT, 8], U32, tag="idxs")
            vout = outp.tile([P, NT, D], F32, tag="vout")
            # big/small interleave: PE stays fed with large tiles while the
            # vector engine's backlog drains on small ones; the pair ends on
            # the cheapest tiles so the end-of-kernel tail is short.
            order = []
            lo_m, hi_m = 0, NT - 1
            while hi_m >= lo_m:
                order.append(hi_m); hi_m -= 1
                if hi_m >= lo_m:
                    order.append(lo_m); lo_m += 1
            # order = [15, 0, 14, 1, ..., 8, 7]
            for m in order:
                W = (m + 1) * P
                ms = slice(m * P, (m + 1) * P)
                S = work.tile([P, N], F32, tag="S")
                nchunks = (W + 511) // 512
                for c in range(nchunks):
                    lo = c * 512
                    hi = min(W, lo + 512)
                    ps = ps_pool.tile([P, 512], F32, tag="ps")
                    nc.tensor.matmul(ps[:, : hi - lo], lhsT=qhl[:, ms],
                                     rhs=khh[:, lo:hi], start=True, stop=False)
                    if hi == W:
                        # causal mask on the diagonal 128 cols via the PE:
                        # adds MASKVAL*[j > i] mid-accumulation-group.
                        nc.tensor.matmul(ps[:, W - P - lo:W - lo], lhsT=triA,
                                         rhs=negI, start=False, stop=False)
                    nc.tensor.matmul(ps[:, : hi - lo], lhsT=qho[:, ms],
                                     rhs=klb[:, lo:hi], start=False, stop=True)
                    nc.scalar.copy(S[:, lo:hi], ps[:, : hi - lo])
                mx = small.tile([P, 8], F32, tag="mx")
                nc.vector.max(mx, S[:, 0:W])
                nc.vector.max_index(idxs[:, m, :], mx, S[:, 0:W])
                # gather the 128 winning v rows for this row tile.
                # NB: one offset column per indirect DMA — multi-column offset
                # tables mis-generate descriptors on HW.
                nc.gpsimd.indirect_dma_start(
                    out=vout[:, m, :],
                    out_offset=None,
                    in_=v,
                    in_offset=bass.IndirectOffsetOnAxis(ap=idxs[:, m, 0:1], axis=1),
                    element_offset=t * N * D,
                )

            # two half-writes: the m=15..8 gathers finish long before m=7..0
            nc.sync.dma_start(out=out[t][:, 8:NT, :], in_=vout[:, 8:NT, :])
            nc.sync.dma_start(out=out[t][:, 0:8, :], in_=vout[:, 0:8, :])


_NC_CACHE = None


def build_nc():
    global _NC_CACHE
    if _NC_CACHE is not None:
        return _NC_CACHE
    nc = bacc.Bacc(
        "TRN2",
        target_bir_lowering=False,
        debug=False,
        enable_asserts=False,
        num_devices=NCORES,
    )
    qa = nc.dram_tensor("qa", [T, D + 1, N], F32, kind="ExternalInput").ap()
    kt = nc.dram_tensor("kt", [T, D, N], F32, kind="ExternalInput").ap()
    v = nc.dram_tensor("v", [T, N, D], F32, kind="ExternalInput").ap()
    out = nc.dram_tensor("out", [T, P, NT, D], F32, kind="ExternalOutput").ap()
    with tile.TileContext(nc) as tc:
        kernel_body(tc, qa, kt, v, out)
    nc.compile()
    _NC_CACHE = nc
    return nc


def make_in_maps(q, k, v):
    q = np.asarray(q, dtype=np.float32)
    k = np.asarray(k, dtype=np.float32)
    v = np.asarray(v, dtype=np.float32)
    assert q.shape == (B, H, N, D), q.shape
    in_maps = []
    for c in range(NCORES):
        qa_c = np.empty((T, D + 1, N), np.float32)
        kt_c = np.empty((T, D, N), np.float32)
        v_c = np.empty((T, N, D), np.float32)
        for t in range(T):
            gp = T * c + t
            b, h = divmod(gp, H)
            qa_c[t, :D] = q[b, h].T
            qa_c[t, D] = 1.0
            kt_c[t] = k[b, h].T
            v_c[t] = v[b, h]
        in_maps.append({"qa": qa_c, "kt": kt_c, "v": v_c})
    return in_maps


def unmarshal(results):
    out = np.empty((B, H, N, D), np.float32)
    for c in range(NCORES):
        o = np.asarray(results[c]["out"])  # [T, P, NT, D]
        for t in range(T):
            gp = T * c + t
            b, h = divmod(gp, H)
            out[b, h] = o[t].transpose(1, 0, 2).reshape(N, D)
    return out


def kernel(q, k, v):
    nc = build_nc()
    in_maps = make_in_maps(q, k, v)
    res = bass_utils.run_bass_kernel_spmd(nc, in_maps, core_ids=list(range(NCORES)))
    return unmarshal(res.results)
